# revision 34
# baseline (speedup 1.0000x reference)
"""Trainium2 Bass kernel for SAGAN-style self-attention with spectral-norm 1x1 convs.

Reference computation (per batch element b, with N = H*W = 4096 spatial
positions, C = 256 channels, D = 32 attention dim):
    f = x @ kf + bf ; g = x @ kg + bg ; h = x @ kh + bh      (kX spectrally normalized)
    S = g @ f^T ; beta = softmax(S, axis=-1)
    v = beta @ h ; out = gamma * (v @ kv + bv) + x

Device strategy (data-parallel: one batch element per NeuronCore, 8 cores):
  - Host: power-iteration spectral norm of the tiny weights (fp64), plus folds:
      * bf drops out of softmax entirely (adds a per-query constant to S).
      * bh is folded into the output projection bias: bv' = bh @ kv + bv.
      * gamma is folded into the output projection: kva = [gamma*kv ; gamma*bv'].
  - Device, per core: the kernel is exp-evacuation bound. The N^2 = 16.7M
    scores land in PSUM fp32 (TRN2 matmul output is always fp32) and must be
    exp'd out at ~1 element/cycle/partition by the two PSUM-capable engines.
    HW microbenches (work/mb.py, loop-slope via work/bench2.py) measured:
      * ScalarE exp gang [128,1536] back-to-back: 1962 ns (= 1573 op + 389
        inter-op gap). Per-op overhead is ~683 ns -> bigger gangs would
        save up to ~20 us but sz-3 x 2 bufs is PSUM-bank-optimal (sz-4
        needs 8 banks for the gang rotation alone).
      * DVE Schraudolph tensor_scalar [128,512] fp32-PSUM -> int16-bitcast
        bf16 SBUF: 809 ns/tile with a 2-bank ping-pong, 1471 ns 1-bank.
      * ScalarE + DVE reading PSUM concurrently (different banks): each
        DVE-PSUM tile slows the ScalarE stream ~340 ns — the engines share
        a PSUM read path (the same DVE ops reading SBUF add ZERO
        interference). So DVE exp offload nets only ~200-300 ns/tile and
        only if its PE fills never block the in-order PE queue.
    Shipped config: 32 key-tiles per query block split into sz-3 ScalarE
    gangs (sgp: 2 bufs x 3 banks) + K_VT=5 single-tile DVE Schraudolph
    exps through a 1-buf vgp bank (Bresenham-spread between S-packs; the
    vgp bank is shared ONLY with the epilogue's tiny rr broadcast — adding
    op_ps to that rotation measured +13 us).
    Schraudolph end-to-end accuracy is a non-issue here (work/acc_sim.py:
    softmax is near-one-hot at score std ~5.9; rel err stays ~2e-5 even
    with ALL tiles on fast-exp; shipped rel err 6.0e-4, gate 2e-2).
  - Query blocks processed in PAIRS sharing one PSUM accumulator bank
    (attention output + softmax denominators via a ones row in haug);
    PV lags K_LAG pushes behind QK/exp. Epilogues are STAGED (K_EPIA=1):
    the DVE-only 1/Z chain (Zs copy + reciprocal_approx_fast) is emitted
    the moment a pair's last PV lands, and the PE-touching remainder
    (broadcast matmul, normalize, projection, residual) is deferred K_EPID
    pushes — so the PE queue never waits on the recip chain. Staging
    measured -8 us.
  - op_ps (output projection PSUM tiles) rotate in the O-accumulator bank
    ("o" tag) — a clean forward chain O -> va -> op_ps -> O(next pair).
    K_OP2=1 packs TWO [128,C] projections per [128,2C] tile (op_ps is only
    half a bank), halving the PE<->DVE ping-pongs, DVE drains, and output
    DMAs in the chain: measured -6.7 us.
  - PSUM banks: sgp 2x3 + vgp 1 + oap 1 = 8 (full).
  - Measured (min-of-8 loop-slope): this config 171.8 us (without K_OP2:
    178.5; K_VT=4: 179.9; K_VT=6 with K_OP2: 175.6); previous-session
    baseline (all-ScalarE sz-3 gangs, separate opp bank): 188 us.
  - Negative results (all HW-measured): sz-2 gangs all-ScalarE +39 us
    (per-gang overhead dominates); any K_VT >= 6 with vgp shared by op_ps
    +8..+20 us (V-stream couples to epilogue drains); sz-2 gangs + 2-buf
    or 3-buf vgp V-streams at K_VT=8..22 all regress (V-pack PE fills
    block the in-order PE queue; ScalarE starves).
"""

import os
import sys

import numpy as np

try:
    import concourse.bass as bass  # noqa: F401
except Exception:  # pragma: no cover - path setup for fresh environments
    for _p in ("/opt/trn_rl_repo", "/root/.axon_site/_ro/trn_rl_repo"):
        if os.path.isdir(_p) and _p not in sys.path:
            sys.path.insert(0, _p)

B, H, W, C, D = 8, 64, 64, 256, 32
N_FULL = H * W  # 4096

_BUILD_CACHE = {}
LAST_RESULTS = None  # BassKernelResults of the most recent run (for test.py)

# Schraudolph fast-exp constants: bits_bf16(e^x) ~= int16(x * 128/ln2 + c2)
SCH_C1 = 128.0 / float(np.log(2.0))
SCH_C2 = 16250.5

# Engine-assignment knobs (experiment flags; defaults = shipped config)
K_VT = int(os.environ.get("K_VT", "5"))  # key-tiles per qb exp'd on DVE
K_SSZ = int(os.environ.get("K_SSZ", "3"))  # m-tiles per ScalarE gang
K_VB = int(os.environ.get("K_VB", "0"))  # vgp bufs (0 = auto from banks)
K_EPID = int(os.environ.get("K_EPID", "3"))  # pushes to defer epilogues
K_QKROW = int(os.environ.get("K_QKROW", "1"))  # 1 = row-tiled QK
K_LAG = int(os.environ.get("K_LAG", "3"))  # push lag of PV behind QK/exp
K_GPRI = int(os.environ.get("K_GPRI", "1"))  # high-priority gang-release DVE ops
K_FCOPY = os.environ.get("K_FCOPY", "s")  # f^T psum->sbuf copy engine: s|v
K_GCOPY = os.environ.get("K_GCOPY", "v")  # (g+bg)^T copy engine: s|v
K_HAUG = os.environ.get("K_HAUG", "v")  # haug copy engine: s|v
K_RRS = os.environ.get("K_RRS", "v")  # rrs copy engine: s|v
K_EPIA = int(os.environ.get("K_EPIA", "1"))  # 1 = emit Zs+recip at pair end
K_TAPER = int(os.environ.get("K_TAPER", "0"))  # 1 = taper PV lag at the tail
K_TSPL = int(os.environ.get("K_TSPL", "0"))  # 1 = last-pair epilogue side-split
K_XUP = int(os.environ.get("K_XUP", "0"))  # 1 = front-load all xT transposes
K_OP2 = int(os.environ.get("K_OP2", "1"))  # 1 = paired output-projection tiles
K_EPB = int(os.environ.get("K_EPB", "0"))  # 1 = merged-pair 1/Z application
# NOTE: K_EPB is OFF — the side-1 [97,512] broadcast needs a K=1 f32r
# matmul writing dst partition base 64, which the walrus ISA check
# rejects (s3d3_mm_valid_dst_partition), with or without tile_position.


def _knobs():
    return (
        K_VT, K_SSZ, K_VB, K_EPID, K_QKROW, K_LAG, K_GPRI,
        K_FCOPY, K_GCOPY, K_HAUG, K_RRS, K_EPIA, K_TAPER, K_TSPL, K_XUP, K_OP2,
        K_EPB,
    )


def _l2n64(v):
    return v / np.sqrt(np.maximum((v * v).sum(-1, keepdims=True), 1e-12))


def _sn_kernel_host(w, u):
    """Mirror reference._sn_kernel in float64; returns w / sigma in float32."""
    w64 = np.asarray(w, np.float64)
    u64 = np.asarray(u, np.float64)
    wr = w64.reshape(-1, w64.shape[-1])
    v = _l2n64(u64 @ wr.T)
    u2 = _l2n64(v @ wr)
    sigma = ((v @ wr) @ u2.T)[0, 0]
    return (w64 / sigma).astype(np.float32)


def _build_packs(nt, n_vt, ssz):
    """Split the nt key-tiles into ScalarE gangs (ssz tiles) and DVE
    singles (n_vt tiles, Bresenham-spread), ordered by mean key index so
    the two engine streams interleave."""
    vset = {mt for mt in range(nt) if (mt * n_vt) % nt < n_vt}
    stiles = [mt for mt in range(nt) if mt not in vset]
    packs = [("s", stiles[i : i + ssz]) for i in range(0, len(stiles), ssz)]
    packs += [("v", [mt]) for mt in sorted(vset)]
    packs.sort(key=lambda p: sum(p[1]) / len(p[1]))
    return packs


def _build(n, loop_k=1):
    """Build + compile the single-core Bass module for sequence length n.

    loop_k > 1 wraps the whole computation in a hardware loop executing it
    loop_k times — used only for on-device timing (the per-call dispatch
    overhead through the PJRT relay is ~100x the kernel runtime).
    """
    import contextlib

    import concourse.bacc as bacc
    import concourse.mybir as mybir
    import concourse.tile as tile

    f32 = mybir.dt.float32
    f32r = mybir.dt.float32r
    bf16 = mybir.dt.bfloat16
    i16 = mybir.dt.int16
    EXP = mybir.ActivationFunctionType.Exp
    MULT = mybir.AluOpType.mult
    ADD = mybir.AluOpType.add

    NT = n // 128  # number of 128-row key tiles
    NU = n // 512  # number of 512-wide query blocks
    n_vt = min(K_VT, NT - 2) if NT > 2 else 0
    packs = _build_packs(NT, n_vt, K_SSZ)
    # vgp serves the DVE exp stream plus the epilogue's brief rr broadcast
    # (op_ps lives in the O bank instead — sharing vgp with op_ps measured
    # +13 us). Banks: sgp 2*K_SSZ + vgp K_VB + oap 1 <= 8.
    vb = K_VB if K_VB else (8 - 2 * K_SSZ - 1)
    assert 2 * K_SSZ + vb + 1 <= 8, "PSUM bank budget"

    nc = bacc.Bacc(
        "TRN2",
        target_bir_lowering=False,
        debug=False,
        enable_asserts=True,
        num_devices=8,
    )
    xb = nc.dram_tensor("xb", [n, C], f32, kind="ExternalInput").ap()
    xbh = nc.dram_tensor("xbh", [n, C], bf16, kind="ExternalInput").ap()
    kf_d = nc.dram_tensor("kf", [C, D], bf16, kind="ExternalInput").ap()
    kg_d = nc.dram_tensor("kg", [C, D], bf16, kind="ExternalInput").ap()
    kh_d = nc.dram_tensor("kh", [C, D], bf16, kind="ExternalInput").ap()
    bg_d = nc.dram_tensor("bg", [D, 1], f32, kind="ExternalInput").ap()
    kva_d = nc.dram_tensor("kva", [D + 1, C], bf16, kind="ExternalInput").ap()
    y = nc.dram_tensor("y", [n, C], f32, kind="ExternalOutput").ap()

    with tile.TileContext(nc) as tc:
        pools = [
            tc.tile_pool(name="singles", bufs=1),
            tc.tile_pool(name="ptp", bufs=2 * (K_LAG + K_EPID + 3)),
            tc.tile_pool(name="outp", bufs=4),
            tc.tile_pool(name="smallp", bufs=8),
            tc.tile_pool(name="sgp", bufs=2, space="PSUM"),
            tc.tile_pool(name="vgp", bufs=vb, space="PSUM"),
            tc.tile_pool(name="oap", bufs=1, space="PSUM"),
        ]
        with contextlib.ExitStack() as stack:
            singles, ptp, outp, smallp, sgp, vgp, oap = (
                stack.enter_context(p) for p in pools
            )

            # ---------------- constants / inputs in SBUF ----------------
            xrows = singles.tile([128, NT, C], f32)  # x rows: [p, tile, c]
            xT = singles.tile([128, 2, n], bf16)  # x^T: [c%128, c//128, n]
            fTr = singles.tile([128, n], bf16)  # f^T replicated on 4 part-groups
            gTr = singles.tile([128, n], bf16)  # (g+bg)^T replicated
            haug = singles.tile([128, NT * 33], bf16)  # [h | 1] per m-tile
            kf_sb = singles.tile([128, 2, D], bf16)
            kg_sb = singles.tile([128, 2, D], bf16)
            kh_sb = singles.tile([128, 2, D], bf16)
            kva_sb = singles.tile([D + 1, C], bf16)
            bgrep = singles.tile([128, 1], f32)
            ones33 = singles.tile([1, D + 1], f32r)

            xb_t = xb.rearrange("(t p) c -> p t c", p=128)
            nc.sync.dma_start(out=kf_sb, in_=kf_d.rearrange("(ch p) d -> p ch d", p=128))
            nc.sync.dma_start(out=kg_sb, in_=kg_d.rearrange("(ch p) d -> p ch d", p=128))
            nc.sync.dma_start(out=kh_sb, in_=kh_d.rearrange("(ch p) d -> p ch d", p=128))
            nc.sync.dma_start(out=kva_sb, in_=kva_d)
            for j in range(4):
                nc.sync.dma_start(out=bgrep[32 * j : 32 * j + 32, :], in_=bg_d)
            ones33_f = singles.tile([1, D + 1], f32)
            nc.gpsimd.memset(ones33_f, 1.0)
            nc.vector.tensor_copy(out=ones33, in_=ones33_f)
            nc.gpsimd.memset(haug, 1.0)

            # ---------------- emission helpers ----------------
            O_tiles = {}

            def emit_chunk(v):
                """x^T, f^T, (g+bg)^T, h for 1024-wide chunk v (m-tiles 8v..8v+7)."""
                sl = slice(1024 * v, 1024 * (v + 1))
                # x^T via DMA xbar transpose straight from HBM (bf16)
                if not K_XUP:
                    for ch in range(2):
                        nc.sync.dma_start_transpose(
                            out=xT[:, ch, sl],
                            in_=xbh[sl, 128 * ch : 128 * (ch + 1)],
                        )
                # f^T / (g+bg)^T projected into all 4 partition groups via
                # concurrent column-tiled matmuls; two 512-wide halves to fit
                # the [128,1024] gang slot.
                for w_sb, dst, bias in ((kf_sb, fTr, None), (kg_sb, gTr, bgrep)):
                    psR = sgp.tile([128, 1024], f32, tag="sg", name="psR")
                    for h2 in range(2):
                        for ch in range(2):
                            for j in range(4):
                                nc.tensor.matmul(
                                    out=psR[32 * j : 32 * (j + 1), 512 * h2 : 512 * (h2 + 1)],
                                    lhsT=w_sb[:, ch, :],
                                    rhs=xT[:, ch, 1024 * v + 512 * h2 : 1024 * v + 512 * (h2 + 1)],
                                    start=(ch == 0),
                                    stop=(ch == 1),
                                    tile_position=(0, 32 * j),
                                )
                    if bias is None:
                        # f copy: ScalarE or (gang-priority) VectorE. An
                        # UNPRIORITIZED DVE copy measured +37us (it waits in
                        # the DVE queue while holding a gang buffer).
                        if K_FCOPY == "s":
                            nc.scalar.copy(out=dst[:, sl], in_=psR)
                        else:
                            import contextlib as _cl

                            with tc.high_priority() if K_GPRI else _cl.nullcontext():
                                nc.vector.tensor_copy(out=dst[:, sl], in_=psR)
                    elif K_GCOPY == "s":
                        # per-partition bias add on ScalarE (Identity + bias AP)
                        nc.scalar.activation(
                            out=dst[:, sl],
                            in_=psR,
                            func=mybir.ActivationFunctionType.Identity,
                            bias=bias,
                            scale=1.0,
                        )
                    else:
                        import contextlib as _cl

                        with tc.high_priority() if K_GPRI else _cl.nullcontext():
                            nc.vector.tensor_scalar_add(
                                out=dst[:, sl], in0=psR, scalar1=bias
                            )
                # h rows for the 8 m-tiles, batched into one PSUM bank
                hp = sgp.tile([128, 8 * D], f32, tag="sg", name="hp")
                for k in range(8):
                    t = 8 * v + k
                    for ch in range(2):
                        nc.tensor.matmul(
                            out=hp[:, D * k : D * (k + 1)],
                            lhsT=xT[:, ch, 128 * t : 128 * (t + 1)],
                            rhs=kh_sb[:, ch, :],
                            start=(ch == 0),
                            stop=(ch == 1),
                        )
                _hcp = nc.scalar.copy if K_HAUG == "s" else (
                    lambda out, in_: nc.vector.tensor_copy(out=out, in_=in_)
                )
                import contextlib as _cl

                with tc.high_priority() if K_GPRI else _cl.nullcontext():
                    _hcp(
                        out=haug[:, 33 * 8 * v : 33 * 8 * (v + 1)].rearrange(
                            "p (k e) -> p k e", k=8
                        )[:, :, 0:D],
                        in_=hp.rearrange("p (k d) -> p k d", k=8),
                    )

            def emit_qk_exp(qb, pi):
                kind, tiles = packs[pi]
                qs = slice(512 * qb, 512 * (qb + 1))
                sz = len(tiles)
                if kind == "s":
                    sg = sgp.tile([128, 512 * sz], f32, tag="sg", name="sg")
                    for i, mt in enumerate(tiles):
                        r = 32 * i if K_QKROW else 0
                        nc.tensor.matmul(
                            out=sg[:, 512 * i : 512 * (i + 1)],
                            lhsT=fTr[r : r + 32, 128 * mt : 128 * (mt + 1)],
                            rhs=gTr[r : r + 32, qs],
                            start=True,
                            stop=True,
                        )
                    pt = ptp.tile([128, 512 * sz], bf16, tag="spt", name="pt")
                    nc.scalar.activation(out=pt, in_=sg, func=EXP)
                    return pt
                # V-pack: single tile, QK on PE row strips 2/3, Schraudolph
                # fast-exp on DVE (int16 bit trick; accuracy verified in
                # work/acc_sim.py — softmax here is near-one-hot).
                # PE input base partition must be 0/32/64 — strip 3 (96) is
                # rejected, so all V-pack QKs share row strip 2.
                mt = tiles[0]
                r = 64 if K_QKROW else 0
                vg = vgp.tile([128, 512], f32, tag="vg", name="vg")
                nc.tensor.matmul(
                    out=vg,
                    lhsT=fTr[r : r + 32, 128 * mt : 128 * (mt + 1)],
                    rhs=gTr[r : r + 32, qs],
                    start=True,
                    stop=True,
                )
                pt = ptp.tile([128, 512], bf16, tag="vpt", name="ptv")
                import contextlib as _cl

                with tc.high_priority() if K_GPRI else _cl.nullcontext():
                    with nc.allow_low_precision(reason="Schraudolph fast-exp bits"):
                        nc.vector.tensor_scalar(
                            out=pt.bitcast(i16),
                            in0=vg,
                            scalar1=SCH_C1,
                            scalar2=SCH_C2,
                            op0=MULT,
                            op1=ADD,
                        )
                return pt

            def emit_pv_pair(pr, pi, pta, ptb):
                """PV for both qbs of pair pr: side 0 <- qb 2*pr (col strips
                0-1), side 1 <- qb 2*pr+1 (col strips 2-3), interleaved so the
                two column-tiles stream concurrently."""
                kind, tiles = packs[pi]
                O = O_tiles[pr]
                last = pi == len(packs) - 1
                for i, mt in enumerate(tiles):
                    for side, pt in ((0, pta), (1, ptb)):
                        nc.tensor.matmul(
                            out=O[64 * side : 64 * side + 33, :],
                            lhsT=haug[:, 33 * mt : 33 * mt + 33],
                            rhs=pt[:, 512 * i : 512 * (i + 1)],
                            start=(pi == 0 and i == 0),
                            stop=(last and i == len(tiles) - 1),
                            tile_position=(0, 64 * side),
                        )

            def emit_epi_a(pr, side):
                """Stage A: 1/Z chain (DVE-only; no PE ops) — can run as soon
                as the pair's last PV lands, hiding the recip latency from
                the PE queue."""
                O = O_tiles[pr]
                base = 64 * side
                from concourse.dve_ops import (
                    RECIP_APPROX_FAST_CONSTS as _RC,
                    RECIPROCAL_APPROX_FAST as _RF,
                )

                Zs = smallp.tile([1, 512], f32, name="Zs")
                nc.vector.tensor_copy(out=Zs, in_=O[base + 32 : base + 33, :])
                rs = smallp.tile([1, 512], f32r, name="rs")
                with nc.allow_low_precision(reason="1/Z at ~51 ULP"):
                    nc.vector._custom_dve(
                        _RF, out=rs, in0=Zs, s0=_RC["s0"], s1=_RC["s1"], imm2=_RC["imm2"]
                    )
                return rs

            def emit_epi_b(pr, side, rs):
                """Stage B: broadcast 1/Z; normalize; project; residual; store."""
                O = O_tiles[pr]
                base = 64 * side
                rr = vgp.tile([D + 1, 512], f32, tag="vg", name="rr")
                nc.tensor.matmul(out=rr, lhsT=ones33, rhs=rs, start=True, stop=True)
                rrs = smallp.tile([D + 1, 512], bf16, name="rrs")
                if K_RRS == "s":
                    nc.scalar.copy(out=rrs, in_=rr)
                else:
                    nc.vector.tensor_copy(out=rrs, in_=rr)
                va = smallp.tile([D + 1, 512], bf16, name="va")
                nc.vector.tensor_mul(out=va, in0=O[base : base + 33, :], in1=rrs)
                emit_proj(pr, side, va)

            def emit_epi_b2(pr, rs_pair):
                """Stage B for BOTH sides, merged: one [97,512] broadcast
                tile (rows 0-32 / 64-96 via two K=1 matmuls; rows 33-63 are
                don't-care) + ONE rrs copy + ONE va mul cover both qbs —
                engine ops cost by free-dim, so this halves the DVE ops and
                PSUM-bus reads of the 1/Z application."""
                O = O_tiles[pr]
                rrb = vgp.tile([97, 512], f32, tag="vg", name="rrb")
                for side in range(2):
                    nc.tensor.matmul(
                        out=rrb[64 * side : 64 * side + 33, :],
                        lhsT=ones33,
                        rhs=rs_pair[side],
                        start=True,
                        stop=True,
                    )
                rrs97 = smallp.tile([97, 512], bf16, name="rrs97")
                if K_RRS == "s":
                    nc.scalar.copy(out=rrs97, in_=rrb)
                else:
                    nc.vector.tensor_copy(out=rrs97, in_=rrb)
                va97 = smallp.tile([97, 512], bf16, name="va97")
                nc.vector.tensor_mul(out=va97, in0=O[0:97, :], in1=rrs97)
                for side in range(2):
                    emit_proj(pr, side, va97[64 * side : 64 * side + 33, :])

            def emit_proj(pr, side, va):
                """Output projection + residual + store for one qb."""
                qb = 2 * pr + side
                # op_ps rotates in the O-accumulator bank: it is written
                # only after va has consumed O, so the 1-buf "o" tag
                # rotation O -> op_ps -> O(next pair) is a clean forward
                # chain. Keeping it OUT of vgp leaves that bank dedicated
                # to the DVE exp stream (sharing vgp measured +13 us at
                # K_VT=6). K_OP2 packs TWO [128,C] projections per tile
                # (op_ps is half a bank), halving the PE<->DVE ping-pongs
                # through the rotation. K_TSPL: the LAST pair's side-1
                # chain (pure tail latency) may run in vgp instead so the
                # two sides drain in parallel.
                def _ops_tile(width):
                    if K_TSPL and side == 1 and pr == NPAIR - 1:
                        return vgp.tile([128, width], f32, tag="vg", name="op_ps")
                    return oap.tile([128, width], f32, tag="o", name="op_ps")

                if K_OP2:
                    for j2 in range(2):
                        nt0 = 4 * qb + 2 * j2
                        op_ps = _ops_tile(2 * C)
                        for dj in range(2):
                            nc.tensor.matmul(
                                out=op_ps[:, C * dj : C * (dj + 1)],
                                lhsT=va[:, 128 * (2 * j2 + dj) : 128 * (2 * j2 + dj + 1)],
                                rhs=kva_sb,
                                start=True,
                                stop=True,
                            )
                        ot = outp.tile([128, 2 * C], f32, name="ot", tag="ot2")
                        nc.vector.tensor_add(
                            out=ot,
                            in0=op_ps,
                            in1=xrows[:, nt0 : nt0 + 2, :].rearrange("p t c -> p (t c)"),
                        )
                        nc.sync.dma_start(
                            out=y[128 * nt0 : 128 * (nt0 + 2), :].rearrange(
                                "(t p) c -> p t c", p=128
                            ),
                            in_=ot.rearrange("p (t c) -> p t c", t=2),
                        )
                else:
                    for j in range(4):
                        nt = 4 * qb + j
                        op_ps = _ops_tile(C)
                        nc.tensor.matmul(
                            out=op_ps,
                            lhsT=va[:, 128 * j : 128 * (j + 1)],
                            rhs=kva_sb,
                            start=True,
                            stop=True,
                        )
                        ot = outp.tile([128, C], f32, name="ot")
                        nc.vector.tensor_add(out=ot, in0=op_ps, in1=xrows[:, nt, :])
                        nc.sync.dma_start(out=y[128 * nt : 128 * (nt + 1), :], in_=ot)

            def emit_epilogue(pr, side):
                emit_epi_b(pr, side, emit_epi_a(pr, side))

            # ---------------- emission schedule ----------------
            # Prologue chunks interleaved with qb0's packs as their f/h
            # tiles become available, so the exp engines start early.
            loop_cm = (
                tc.For_i(0, loop_k, 1, name="rep")
                if loop_k > 1
                else contextlib.nullcontext()
            )
            loop_cm.__enter__()
            for v in range(NT // 8):
                nc.sync.dma_start(
                    out=xrows[:, 8 * v : 8 * (v + 1), :],
                    in_=xb_t[:, 8 * v : 8 * (v + 1), :],
                )
            if K_XUP:
                # issue every chunk's x^T xbar transpose up front: the DMA
                # queues drain them during the early gangs, so later chunks'
                # f/g projections never wait on a fresh transpose
                for v in range(NT // 8):
                    for ch in range(2):
                        nc.sync.dma_start_transpose(
                            out=xT[:, ch, 1024 * v : 1024 * (v + 1)],
                            in_=xbh[1024 * v : 1024 * (v + 1), 128 * ch : 128 * (ch + 1)],
                        )

            # qbs are processed in PAIRS: qb 2p on column strips 0-1, qb 2p+1
            # on strips 2-3 — the two PV column-tiles stream concurrently.
            # The PV stream lags LAG pushes behind QK/exp, and epilogues are
            # deferred K_EPID pushes past the pair's last pack so the
            # epilogue's PE ops (which wait on the DVE 1/Z chain) sit behind
            # fresh QK packs in the PE queue instead of starving the exps.
            from collections import deque

            assert NU % 2 == 0, "pair scheduling needs an even number of qbs"
            NPAIR = NU // 2
            LAG = K_LAG if len(packs) > K_LAG + 1 else 0
            pend = {}
            epi_rs = {}
            state = {"outstanding": 0, "cur": 0, "epi": None, "cd": 0, "lag": LAG}

            def push_pair(pr, pi):
                a = emit_qk_exp(2 * pr, pi)
                b = emit_qk_exp(2 * pr + 1, pi)
                pend.setdefault(pr, deque()).append((pi, a, b))
                state["outstanding"] += 1
                if state["cd"] > 0:
                    state["cd"] -= 1

            def pop_pv(force=False):
                while True:
                    if state["epi"] is not None:
                        if state["cd"] > 0 and not force:
                            return
                        pr = state["epi"]
                        if K_EPIA:
                            rsp = (epi_rs.pop((pr, 0)), epi_rs.pop((pr, 1)))
                            if K_EPB:
                                emit_epi_b2(pr, rsp)
                            else:
                                emit_epi_b(pr, 0, rsp[0])
                                emit_epi_b(pr, 1, rsp[1])
                        elif K_EPB:
                            rsp = (emit_epi_a(pr, 0), emit_epi_a(pr, 1))
                            emit_epi_b2(pr, rsp)
                        else:
                            emit_epilogue(pr, 0)
                            emit_epilogue(pr, 1)
                        state["epi"] = None
                        state["cur"] += 1
                    if state["outstanding"] <= (0 if force else state["lag"]):
                        return
                    pr = state["cur"]
                    if not pend.get(pr):
                        return
                    pi, pta, ptb = pend[pr].popleft()
                    state["outstanding"] -= 1
                    if pi == 0:
                        O_tiles[pr] = oap.tile([128, 512], f32, tag="o", name="O")
                    emit_pv_pair(pr, pi, pta, ptb)
                    if pi == len(packs) - 1:
                        state["epi"] = pr
                        state["cd"] = K_EPID
                        if K_EPIA:
                            epi_rs[(pr, 0)] = emit_epi_a(pr, 0)
                            epi_rs[(pr, 1)] = emit_epi_a(pr, 1)
                    if not force:
                        return

            next_p0 = 0
            for v in range(NT // 8):
                emit_chunk(v)
                while next_p0 < len(packs) and (
                    max(packs[next_p0][1]) <= 8 * v + 7
                ):
                    push_pair(0, next_p0)
                    next_p0 += 1
                    pop_pv()
            for pr in range(NPAIR):
                for pi in range(next_p0 if pr == 0 else 0, len(packs)):
                    push_pair(pr, pi)
                    pop_pv()
                    if K_TAPER and pr == NPAIR - 1 and pi >= len(packs) - LAG - 1:
                        # tail taper: drain the PV lag during the last packs
                        # so less work remains after the final exp
                        state["lag"] = max(1, state["lag"] - 1)
                        pop_pv()
            pop_pv(force=True)
            loop_cm.__exit__(None, None, None)

    nc.compile()
    return nc


def _prep_weights(wf, uf, wg, ug, wh, uh, wv, uv, bh, bv, gamma):
    kf = _sn_kernel_host(wf, uf)
    kg = _sn_kernel_host(wg, ug)
    kh = _sn_kernel_host(wh, uh)
    kv = _sn_kernel_host(wv, uv)
    gamma = float(np.asarray(gamma, np.float64)[0])
    bvp = np.asarray(bh, np.float64) @ np.asarray(kv, np.float64) + np.asarray(
        bv, np.float64
    )
    kva = np.concatenate(
        [gamma * np.asarray(kv, np.float64), (gamma * bvp)[None, :]], axis=0
    ).astype(np.float32)
    return kf, kg, kh, kva


def kernel(
    x, wf, bf, uf, wg, bg, ug, wh, bh, uh, wv, bv, uv, gamma, _n=None, _loop_k=1
) -> np.ndarray:
    global LAST_RESULTS
    from concourse import bass_utils

    n = _n or N_FULL
    _key = (n, _loop_k, _knobs())
    if _key not in _BUILD_CACHE:
        _BUILD_CACHE[_key] = _build(n, loop_k=_loop_k)
    nc = _BUILD_CACHE[_key]

    import ml_dtypes

    kf, kg, kh, kva = _prep_weights(wf, uf, wg, ug, wh, uh, wv, uv, bh, bv, gamma)
    bg2 = np.ascontiguousarray(np.asarray(bg, np.float32).reshape(D, 1))
    bfd = ml_dtypes.bfloat16
    kf, kg, kh, kva = (np.ascontiguousarray(a.astype(bfd)) for a in (kf, kg, kh, kva))

    x = np.asarray(x, np.float32)
    b = x.shape[0]
    xflat = np.ascontiguousarray(x.reshape(b, -1, C)[:, :n, :])
    xflat_bf = np.ascontiguousarray(xflat.astype(bfd))
    in_maps = [
        {
            "xb": np.ascontiguousarray(xflat[i]),
            "xbh": xflat_bf[i],
            "kf": kf,
            "kg": kg,
            "kh": kh,
            "bg": bg2,
            "kva": kva,
        }
        for i in range(b)
    ]

    trace = bool(int(os.environ.get("BASS_KERNEL_TRACE", "0")))
    try:
        LAST_RESULTS = bass_utils.run_bass_kernel_spmd(
            nc,
            in_maps,
            core_ids=list(range(b)),
            trace=trace,
            trace_cores=[0] if trace else None,
        )
    except ModuleNotFoundError:
        # NTFF profiling hook unavailable in this environment
        LAST_RESULTS = bass_utils.run_bass_kernel_spmd(
            nc, in_maps, core_ids=list(range(b))
        )
    out = np.stack([r["y"] for r in LAST_RESULTS.results], axis=0)
    if n == N_FULL:
        out = out.reshape(b, H, W, C)
    return out
